# revision 2
# baseline (speedup 1.0000x reference)
"""Grouped-query attention (B=8,S=512,D=4096,G=32) on 8 trn2 cores.

Data-parallel over batch: core b handles batch b. Per core everything is
computed in a feature-major layout (no on-device transposes):

  qT[f,t] accumulates 32 matmuls of wq-tile.T @ x-tile per 128-feature
  block (= per head); same for kT; v token-major. RoPE on heads 0..7
  (per-token scalar angle; head g pairs with g+4). ALPHA is folded into
  xq on the host. Attention per head: sT = k_blk.T @ qT, exp via ACT
  (mask as bias), o accumulated from v blocks; softmax denominators for
  all 32 heads accumulate into one [32,512] PSUM tile via one-hot
  matmuls -> single reciprocal -> per-head rank-1 broadcast matmul ->
  in-place normalize. Unnormalized o overwrites v's SBUF space (head g's
  v is dead once its o-matmuls issued). Output projection token-major.

All weight/activation DRAM parameters are host-pre-tiled so every DMA is
one fully contiguous >=1MiB transfer.
"""

import math

import numpy as np
import ml_dtypes

import concourse.bass as bass
import concourse.mybir as mybir
import concourse.tile as tile
from concourse import bacc
from concourse.bass_utils import run_bass_kernel_spmd

B, S, D = 8, 512, 4096
G, DH = 32, 128
RD = 1024
ALPHA = 1.0 / math.sqrt(DH)
PI = math.pi
NCORES = 8
DT = mybir.dt
AF = mybir.ActivationFunctionType
ALU = mybir.AluOpType

# set by test.py to capture a profile
TRACE = False
LAST_RESULT = None


def _range_reduce(nc, ang, mtmp):
    """In-place reduce ang (>=0, < ~8*pi) into (-pi, pi] mod 2*pi."""
    for _ in range(4):
        nc.vector.tensor_scalar(mtmp, ang, PI, 2.0 * PI, ALU.is_gt, ALU.mult)
        nc.vector.tensor_sub(ang, ang, mtmp)


def build_program():
    nc = bacc.Bacc(
        "TRN2", target_bir_lowering=False, debug=False, num_devices=NCORES
    )
    bf16 = DT.bfloat16
    f32 = DT.float32

    xq_d = nc.declare_dram_parameter("xq", [128, 32, S], bf16, isOutput=False)
    xk_d = nc.declare_dram_parameter("xk", [128, 32, S], bf16, isOutput=False)
    xv_d = nc.declare_dram_parameter("xv", [128, 32, S], bf16, isOutput=False)
    wq_d = nc.declare_dram_parameter(
        "wq", [32, 128, 32, 128], bf16, isOutput=False
    )
    wk_d = nc.declare_dram_parameter(
        "wk", [32, 128, 32, 128], bf16, isOutput=False
    )
    wv_d = nc.declare_dram_parameter(
        "wv", [8, 128, 32, 512], bf16, isOutput=False
    )
    wo_d = nc.declare_dram_parameter(
        "wo", [8, 128, 32, 512], bf16, isOutput=False
    )
    pos_d = nc.declare_dram_parameter("pos", [S], f32, isOutput=False)
    invf_d = nc.declare_dram_parameter("invf", [S], f32, isOutput=False)
    mask_d = nc.declare_dram_parameter("maskin", [S], DT.int32, isOutput=False)
    ssel_d = nc.declare_dram_parameter(
        "ssel", [128, G * 32], bf16, isOutput=False
    )
    bsel_d = nc.declare_dram_parameter(
        "bsel", [32, G * 128], bf16, isOutput=False
    )
    y_d = nc.declare_dram_parameter("y", [32, 128, 512], f32, isOutput=True)

    with tile.TileContext(nc) as tc:
        with tc.tile_pool(name="persist", bufs=1) as persist:
            sin_t = persist.tile([128, S], f32, tag="sin")
            cos_t = persist.tile([128, S], f32, tag="cos")
            maskb = persist.tile([128, 4], f32, tag="maskb")
            ones_f1 = persist.tile([1, 128], f32, tag="ones_f1")
            ssel_s = persist.tile([128, G * 32], bf16, tag="ssel")
            bsel_s = persist.tile([32, G * 128], bf16, tag="bsel")
            qT_s = persist.tile([128, G, S], bf16, tag="qT")
            kT_s = persist.tile([128, G, S], bf16, tag="kT")
            # v during attention; overwritten per head by unnormalized oT
            # (stored so that v_s[:, tb, g, :] == attnT[g][:, tb*128:+128])
            v_s = persist.tile([128, 4, G, DH], bf16, tag="v")

            nc.vector.memset(ones_f1, 1.0)
            nc.sync.dma_start(out=ssel_s, in_=ssel_d[:, :])
            nc.sync.dma_start(out=bsel_s, in_=bsel_d[:, :])

            # ---- setup: trig + mask ----
            with (
                tc.tile_pool(name="setup", bufs=1) as setup,
                tc.tile_pool(name="ps_setup", bufs=2, space="PSUM") as ps_setup,
            ):
                pos1 = setup.tile([1, S], f32, tag="pos1")
                invf1 = setup.tile([1, S], f32, tag="invf1")
                angc = setup.tile([1, S], f32, tag="angc")
                mtmp = setup.tile([1, S], f32, tag="mtmp")
                mi = setup.tile([128, 4], DT.int32, tag="mi")
                mf = setup.tile([128, 4], f32, tag="mf")

                nc.sync.dma_start(out=pos1, in_=pos_d[None, :])
                nc.sync.dma_start(out=invf1, in_=invf_d[None, :])
                angs = pos1
                nc.vector.tensor_mul(angs, pos1, invf1)
                nc.vector.tensor_scalar_add(angc, angs, PI / 2.0)
                _range_reduce(nc, angs, mtmp)
                _range_reduce(nc, angc, mtmp)
                nc.scalar.activation(angs, angs, AF.Sin)
                nc.scalar.activation(angc, angc, AF.Sin)
                ps_sin = ps_setup.tile([128, S], f32, tag="b", name="ps_sin")
                ps_cos = ps_setup.tile([128, S], f32, tag="b", name="ps_cos")
                nc.tensor.matmul(ps_sin, ones_f1, angs, start=True, stop=True)
                nc.tensor.matmul(ps_cos, ones_f1, angc, start=True, stop=True)
                nc.scalar.copy(sin_t, ps_sin)
                nc.scalar.copy(cos_t, ps_cos)

                nc.sync.dma_start(
                    out=mi, in_=mask_d[:].rearrange("(b p) -> p b", p=128)
                )
                nc.vector.tensor_copy(mf, mi)
                nc.vector.tensor_scalar(
                    maskb, mf, 1.0e9, 1.0e9, ALU.mult, ALU.subtract
                )

            # ---- q and k projections (feature-major out) + RoPE ----
            # fb == head index (DH == 128). RoPE pairs (g, g+4), g<4.
            fb_order = [0, 4, 1, 5, 2, 6, 3, 7] + list(range(8, 32))
            for which, w_d, x_d, outT in (
                ("q", wq_d, xq_d, qT_s),
                ("k", wk_d, xk_d, kT_s),
            ):
                with (
                    tc.tile_pool(name=f"x{which}", bufs=1) as xin_pool,
                    tc.tile_pool(name=f"w{which}", bufs=2) as w_pool,
                    tc.tile_pool(name=f"ps{which}", bufs=4, space="PSUM") as ps_pool,
                    tc.tile_pool(name=f"rt{which}", bufs=4) as rtmp_pool,
                ):
                    x_s = xin_pool.tile([128, 32, S], bf16, tag="x")
                    nc.sync.dma_start(out=x_s, in_=x_d[:, :, :])
                    pair_ps = {}
                    for fb in fb_order:
                        wt = w_pool.tile([128, 32, 128], bf16, tag="w")
                        nc.sync.dma_start(out=wt, in_=w_d[fb, :, :, :])
                        ps = ps_pool.tile([128, S], f32, tag="ps", name="ps_qk")
                        for db in range(32):
                            nc.tensor.matmul(
                                ps,
                                wt[:, db, :],
                                x_s[:, db, :],
                                start=(db == 0),
                                stop=(db == 31),
                            )
                        if fb >= 8:
                            nc.vector.tensor_copy(outT[:, fb, :], ps)
                        elif fb < 4:
                            pair_ps[fb] = ps
                        else:
                            g = fb - 4
                            ps_a, ps_b = pair_ps.pop(g), ps
                            t1 = rtmp_pool.tile([128, S], f32, tag="t")
                            t2 = rtmp_pool.tile([128, S], f32, tag="t")
                            t3 = rtmp_pool.tile([128, S], f32, tag="t")
                            t4 = rtmp_pool.tile([128, S], f32, tag="t")
                            nc.vector.tensor_mul(t1, ps_a, cos_t)
                            nc.vector.tensor_mul(t2, ps_b, sin_t)
                            nc.vector.tensor_sub(outT[:, g, :], t1, t2)
                            nc.vector.tensor_mul(t3, ps_b, cos_t)
                            nc.vector.tensor_mul(t4, ps_a, sin_t)
                            nc.vector.tensor_add(outT[:, g + 4, :], t3, t4)

            # ---- v projection (token-major) ----
            with (
                tc.tile_pool(name="xv", bufs=1) as xin_pool,
                tc.tile_pool(name="wv", bufs=3) as w_pool,
                tc.tile_pool(name="psv", bufs=8, space="PSUM") as ps_pool,
            ):
                x_s = xin_pool.tile([128, 32, S], bf16, tag="x")
                nc.sync.dma_start(out=x_s, in_=xv_d[:, :, :])
                for fc in range(8):
                    wlo = w_pool.tile([128, 16, 512], bf16, tag="w")
                    whi = w_pool.tile([128, 16, 512], bf16, tag="w")
                    nc.sync.dma_start(out=wlo, in_=wv_d[fc, :, :16, :])
                    nc.sync.dma_start(out=whi, in_=wv_d[fc, :, 16:, :])
                    pss = [
                        ps_pool.tile([128, 512], f32, tag="ps", name="ps_v")
                        for _ in range(4)
                    ]
                    for db in range(32):
                        wt = wlo if db < 16 else whi
                        for tb in range(4):
                            nc.tensor.matmul(
                                pss[tb],
                                x_s[:, db, tb * 128 : (tb + 1) * 128],
                                wt[:, db % 16, :],
                                start=(db == 0),
                                stop=(db == 31),
                            )
                    for tb in range(4):
                        nc.vector.tensor_copy(
                            v_s[:, tb, fc * 4 : (fc + 1) * 4, :], pss[tb]
                        )

            # ---- attention ----
            with (
                tc.tile_pool(name="wexp", bufs=6) as wexp_pool,
                tc.tile_pool(name="rsm", bufs=1) as rpool,
                tc.tile_pool(name="ps_s", bufs=2, space="PSUM") as ps_s_pool,
                tc.tile_pool(name="ps_o", bufs=2, space="PSUM") as ps_o_pool,
                tc.tile_pool(name="ps_m", bufs=1, space="PSUM") as ps_m_pool,
                tc.tile_pool(name="ps_r", bufs=2, space="PSUM") as ps_r_pool,
            ):
                ps_sums = ps_m_pool.tile([G, S], f32, tag="sums")
                for g in range(G):
                    ps_o = ps_o_pool.tile([128, S], f32, tag="o")
                    for kb in range(4):
                        ps_sc = ps_s_pool.tile([128, S], f32, tag="s")
                        nc.tensor.matmul(
                            ps_sc,
                            kT_s[:, g, kb * 128 : (kb + 1) * 128],
                            qT_s[:, g, :],
                            start=True,
                            stop=True,
                        )
                        wb = wexp_pool.tile([128, S], bf16, tag="w")
                        nc.scalar.activation(
                            wb, ps_sc, AF.Exp, bias=maskb[:, kb : kb + 1], scale=1.0
                        )
                        nc.tensor.matmul(
                            ps_o,
                            v_s[:, kb, g, :],
                            wb,
                            start=(kb == 0),
                            stop=(kb == 3),
                        )
                        nc.tensor.matmul(
                            ps_sums,
                            ssel_s[:, g * 32 : (g + 1) * 32],
                            wb,
                            start=(g == 0 and kb == 0),
                            stop=(g == G - 1 and kb == 3),
                        )
                    # unnormalized oT -> v_s space of head g (v now dead):
                    # v_s[:, tb, g, :] <- ps_o[:, tb*128:(tb+1)*128]
                    nc.vector.tensor_copy(
                        v_s[:, :, g, :],
                        ps_o[:, :].rearrange("p (a b) -> p a b", a=4),
                    )
                # single reciprocal for all heads, then per-head broadcast
                r_all = rpool.tile([G, S], bf16, tag="r")
                with nc.allow_low_precision(
                    reason="softmax reciprocal in bf16 is within tolerance"
                ):
                    nc.vector.reciprocal(r_all, ps_sums)
                for g in range(G):
                    ps_rb = ps_r_pool.tile([128, S], f32, tag="rb")
                    nc.tensor.matmul(
                        ps_rb,
                        bsel_s[:, g * 128 : (g + 1) * 128],
                        r_all,
                        start=True,
                        stop=True,
                    )
                    nc.vector.tensor_mul(
                        v_s[:, :, g, :],
                        v_s[:, :, g, :],
                        ps_rb[:, :].rearrange("p (a b) -> p a b", a=4),
                    )

            # ---- y = attn @ Wo.T (token-major out) ----
            with (
                tc.tile_pool(name="wo", bufs=3) as w_pool,
                tc.tile_pool(name="psy", bufs=8, space="PSUM") as ps_pool,
                tc.tile_pool(name="yout", bufs=4) as y_pool,
            ):
                for fc in range(8):
                    wlo = w_pool.tile([128, 16, 512], bf16, tag="w")
                    whi = w_pool.tile([128, 16, 512], bf16, tag="w")
                    nc.sync.dma_start(out=wlo, in_=wo_d[fc, :, :16, :])
                    nc.sync.dma_start(out=whi, in_=wo_d[fc, :, 16:, :])
                    pss = [
                        ps_pool.tile([128, 512], f32, tag="ps", name="ps_y")
                        for _ in range(4)
                    ]
                    for db in range(32):
                        wt = wlo if db < 16 else whi
                        for tb in range(4):
                            nc.tensor.matmul(
                                pss[tb],
                                v_s[:, tb, db, :],
                                wt[:, db % 16, :],
                                start=(db == 0),
                                stop=(db == 31),
                            )
                    for tb in range(4):
                        yt = y_pool.tile([128, 512], f32, tag="y")
                        nc.vector.tensor_copy(yt, pss[tb])
                        nc.sync.dma_start(out=y_d[fc * 4 + tb, :, :], in_=yt)
    nc.compile()
    return nc


_NC_CACHE = None


def _get_program():
    global _NC_CACHE
    if _NC_CACHE is None:
        _NC_CACHE = build_program()
    return _NC_CACHE


def make_in_maps(query, key, value, mask, position_ids, Wq, Wk, Wv, Wo):
    bf16 = ml_dtypes.bfloat16

    def qk_tile(W):  # [4096,4096] -> [32 fb, 128 p, 32 db, 128 f]
        t = np.asarray(W, np.float32).astype(bf16)
        t = t.reshape(32, 128, 32, 128)  # [fb, f, db, p]
        return np.ascontiguousarray(t.transpose(0, 3, 2, 1))

    def vo_tile(W):  # [4096,4096] -> [8 fc, 128 p, 32 db, 512 f]
        t = np.asarray(W, np.float32).astype(bf16)
        t = t.reshape(8, 512, 32, 128)  # [fc, f, db, p]
        return np.ascontiguousarray(t.transpose(0, 3, 2, 1))

    def x_tile(x, scale=None):  # [512,4096] -> [128 p, 32 db, 512 t]
        x = np.asarray(x, np.float32)
        if scale is not None:
            x = x * scale
        t = x.astype(bf16).T.reshape(32, 128, S)  # [db, p, t]
        return np.ascontiguousarray(t.transpose(1, 0, 2))

    wq = qk_tile(np.asarray(Wq))
    wk = qk_tile(np.asarray(Wk))
    wv = vo_tile(np.asarray(Wv))
    wo = vo_tile(np.asarray(Wo))
    invf = (10000.0 ** (-np.arange(0, RD, 2, dtype=np.float32) / RD)).astype(
        np.float32
    )
    ssel = np.zeros((128, G * 32), bf16)
    for g in range(G):
        ssel[:, g * 32 + g] = 1
    bsel = np.zeros((32, G * 128), bf16)
    for g in range(G):
        bsel[g, g * 128 : (g + 1) * 128] = 1

    in_maps = []
    for b in range(NCORES):
        in_maps.append(
            {
                "xq": x_tile(query[b], ALPHA),
                "xk": x_tile(key[b]),
                "xv": x_tile(value[b]),
                "wq": wq,
                "wk": wk,
                "wv": wv,
                "wo": wo,
                "pos": np.ascontiguousarray(
                    np.asarray(position_ids[b], np.float32)
                ),
                "invf": invf,
                "maskin": np.ascontiguousarray(np.asarray(mask[b], np.int32)),
                "ssel": ssel,
                "bsel": bsel,
            }
        )
    return in_maps


def unshard_y(y_tiles):
    # [32, 128, 512] blocks (fc*4+tb) -> [512, 4096]
    return (
        y_tiles.reshape(8, 4, 128, 512)
        .transpose(1, 2, 0, 3)
        .reshape(S, D)
    )


def kernel(query, key, value, mask, position_ids, Wq, Wk, Wv, Wo):
    global LAST_RESULT
    nc = _get_program()
    in_maps = make_in_maps(
        query, key, value, mask, position_ids, Wq, Wk, Wv, Wo
    )
    res = run_bass_kernel_spmd(
        nc, in_maps, core_ids=list(range(NCORES)), trace=TRACE
    )
    LAST_RESULT = res
    out = np.stack(
        [unshard_y(np.asarray(res.results[b]["y"])) for b in range(NCORES)],
        axis=0,
    )
    return np.ascontiguousarray(out.astype(np.float32))


# revision 3
# speedup vs baseline: 1.0178x; 1.0178x over previous
"""Grouped-query attention (B=8,S=512,D=4096,G=32) on 8 trn2 cores.

Data-parallel over batch: core b handles batch b. Per core everything is
computed in a feature-major layout (no on-device transposes):

  qT[f,t] accumulates 32 matmuls of wq-tile.T @ x-tile per 128-feature
  block (= per head); same for kT; v token-major. RoPE on heads 0..7
  (per-token scalar angle; head g pairs with g+4). ALPHA is folded into
  xq on the host. Attention per head: sT = k_blk.T @ qT, exp via ACT
  (mask as bias), o accumulated from v blocks; softmax denominators for
  all 32 heads accumulate into one [32,512] PSUM tile via one-hot
  matmuls -> single reciprocal -> per-head rank-1 broadcast matmul ->
  in-place normalize. Unnormalized o overwrites v's SBUF space (head g's
  v is dead once its o-matmuls issued). Output projection token-major.

All weight/activation DRAM parameters are host-pre-tiled so every DMA is
one fully contiguous >=1MiB transfer.
"""

import math

import numpy as np
import ml_dtypes

import concourse.bass as bass
import concourse.mybir as mybir
import concourse.tile as tile
from concourse import bacc
from concourse.bass_utils import run_bass_kernel_spmd

B, S, D = 8, 512, 4096
G, DH = 32, 128
RD = 1024
ALPHA = 1.0 / math.sqrt(DH)
PI = math.pi
NCORES = 8
DT = mybir.dt
AF = mybir.ActivationFunctionType
ALU = mybir.AluOpType

# set by test.py to capture a profile
TRACE = False
LAST_RESULT = None


def _range_reduce(nc, ang, mtmp):
    """In-place reduce ang (>=0, < ~8*pi) into (-pi, pi] mod 2*pi."""
    for _ in range(4):
        nc.vector.tensor_scalar(mtmp, ang, PI, 2.0 * PI, ALU.is_gt, ALU.mult)
        nc.vector.tensor_sub(ang, ang, mtmp)


def build_program():
    nc = bacc.Bacc(
        "TRN2", target_bir_lowering=False, debug=False, num_devices=NCORES
    )
    bf16 = DT.bfloat16
    f32 = DT.float32

    xq_d = nc.declare_dram_parameter("xq", [128, 32, S], bf16, isOutput=False)
    xk_d = nc.declare_dram_parameter("xk", [128, 32, S], bf16, isOutput=False)
    xv_d = nc.declare_dram_parameter("xv", [128, 32, S], bf16, isOutput=False)
    wq_d = nc.declare_dram_parameter(
        "wq", [32, 128, 32, 128], bf16, isOutput=False
    )
    wk_d = nc.declare_dram_parameter(
        "wk", [32, 128, 32, 128], bf16, isOutput=False
    )
    wv_d = nc.declare_dram_parameter(
        "wv", [8, 128, 32, 512], bf16, isOutput=False
    )
    wo_d = nc.declare_dram_parameter(
        "wo", [8, 128, 32, 512], bf16, isOutput=False
    )
    pos_d = nc.declare_dram_parameter("pos", [S], f32, isOutput=False)
    invf_d = nc.declare_dram_parameter("invf", [S], f32, isOutput=False)
    mask_d = nc.declare_dram_parameter("maskin", [S], DT.int32, isOutput=False)
    ssel_d = nc.declare_dram_parameter(
        "ssel", [128, 16 * 16], bf16, isOutput=False
    )
    bsel_d = nc.declare_dram_parameter(
        "bsel", [16, 16 * 128], bf16, isOutput=False
    )
    y_d = nc.declare_dram_parameter("y", [32, 128, 512], f32, isOutput=True)

    with tile.TileContext(nc) as tc:
        with tc.tile_pool(name="persist", bufs=1) as persist:
            sin_t = persist.tile([128, S], f32, tag="sin")
            cos_t = persist.tile([128, S], f32, tag="cos")
            maskb = persist.tile([128, 4], f32, tag="maskb")
            ones_f1 = persist.tile([1, 128], f32, tag="ones_f1")
            ssel_s = persist.tile([128, 16 * 16], bf16, tag="ssel")
            bsel_s = persist.tile([16, 16 * 128], bf16, tag="bsel")
            qT_s = persist.tile([128, G, S], bf16, tag="qT")
            kT_s = persist.tile([128, G, S], bf16, tag="kT")
            # v during attention; overwritten per head by unnormalized oT
            # (stored so that v_s[:, tb, g, :] == attnT[g][:, tb*128:+128])
            v_s = persist.tile([128, 4, G, DH], bf16, tag="v")

            nc.vector.memset(ones_f1, 1.0)
            nc.sync.dma_start(out=ssel_s, in_=ssel_d[:, :])
            nc.sync.dma_start(out=bsel_s, in_=bsel_d[:, :])

            # ---- setup: trig + mask ----
            with (
                tc.tile_pool(name="setup", bufs=1) as setup,
                tc.tile_pool(name="ps_setup", bufs=2, space="PSUM") as ps_setup,
            ):
                pos1 = setup.tile([1, S], f32, tag="pos1")
                invf1 = setup.tile([1, S], f32, tag="invf1")
                angc = setup.tile([1, S], f32, tag="angc")
                mtmp = setup.tile([1, S], f32, tag="mtmp")
                mi = setup.tile([128, 4], DT.int32, tag="mi")
                mf = setup.tile([128, 4], f32, tag="mf")

                nc.sync.dma_start(out=pos1, in_=pos_d[None, :])
                nc.sync.dma_start(out=invf1, in_=invf_d[None, :])
                angs = pos1
                nc.vector.tensor_mul(angs, pos1, invf1)
                nc.vector.tensor_scalar_add(angc, angs, PI / 2.0)
                _range_reduce(nc, angs, mtmp)
                _range_reduce(nc, angc, mtmp)
                nc.scalar.activation(angs, angs, AF.Sin)
                nc.scalar.activation(angc, angc, AF.Sin)
                ps_sin = ps_setup.tile([128, S], f32, tag="b", name="ps_sin")
                ps_cos = ps_setup.tile([128, S], f32, tag="b", name="ps_cos")
                nc.tensor.matmul(ps_sin, ones_f1, angs, start=True, stop=True)
                nc.tensor.matmul(ps_cos, ones_f1, angc, start=True, stop=True)
                nc.scalar.copy(sin_t, ps_sin)
                nc.scalar.copy(cos_t, ps_cos)

                nc.sync.dma_start(
                    out=mi, in_=mask_d[:].rearrange("(b p) -> p b", p=128)
                )
                nc.vector.tensor_copy(mf, mi)
                nc.vector.tensor_scalar(
                    maskb, mf, 1.0e9, 1.0e9, ALU.mult, ALU.subtract
                )
                # preload the Exp ACT table set during the projections so
                # attention's first exp doesn't stall on a table switch
                nc.scalar.activation(mtmp, mtmp, AF.Exp, scale=0.0)

            # ---- projections: q, k (feature-major + RoPE) then v ----
            # One x pool (2 slots) spans all three so the next stage's
            # input DMA prefetches during the current stage's matmuls.
            # fb == head index (DH == 128). RoPE pairs (g, g+4), g<4.
            fb_order = [0, 4, 1, 5, 2, 6, 3, 7] + list(range(8, 32))
            with tc.tile_pool(name="xin", bufs=2) as xin_pool:
                xq_s = xin_pool.tile([128, 32, S], bf16, tag="x")
                nc.sync.dma_start(out=xq_s, in_=xq_d[:, :, :])
                xk_s = xin_pool.tile([128, 32, S], bf16, tag="x")
                nc.sync.dma_start(out=xk_s, in_=xk_d[:, :, :])
                for which, w_d, x_s, outT in (
                    ("q", wq_d, xq_s, qT_s),
                    ("k", wk_d, xk_s, kT_s),
                ):
                    with (
                        tc.tile_pool(name=f"w{which}", bufs=2) as w_pool,
                        tc.tile_pool(
                            name=f"ps{which}", bufs=4, space="PSUM"
                        ) as ps_pool,
                        tc.tile_pool(name=f"rt{which}", bufs=2) as rtmp_pool,
                    ):
                        pair_ps = {}
                        for fb in fb_order:
                            wt = w_pool.tile([128, 32, 128], bf16, tag="w")
                            nc.sync.dma_start(out=wt, in_=w_d[fb, :, :, :])
                            ps = ps_pool.tile(
                                [128, S], f32, tag="ps", name="ps_qk"
                            )
                            for db in range(32):
                                nc.tensor.matmul(
                                    ps,
                                    wt[:, db, :],
                                    x_s[:, db, :],
                                    start=(db == 0),
                                    stop=(db == 31),
                                )
                            if fb >= 8:
                                nc.scalar.copy(outT[:, fb, :], ps)
                            elif fb < 4:
                                pair_ps[fb] = ps
                            else:
                                g = fb - 4
                                ps_a, ps_b = pair_ps.pop(g), ps
                                t1 = rtmp_pool.tile([128, S], f32, tag="t")
                                t2 = rtmp_pool.tile([128, S], f32, tag="t")
                                nc.vector.tensor_mul(t1, ps_a, cos_t)
                                nc.vector.tensor_mul(t2, ps_b, sin_t)
                                nc.vector.tensor_sub(outT[:, g, :], t1, t2)
                                t3 = rtmp_pool.tile([128, S], f32, tag="t")
                                t4 = rtmp_pool.tile([128, S], f32, tag="t")
                                nc.vector.tensor_mul(t3, ps_b, cos_t)
                                nc.vector.tensor_mul(t4, ps_a, sin_t)
                                nc.vector.tensor_add(outT[:, g + 4, :], t3, t4)
                    if which == "q":
                        # prefetch xv into the slot xq_s releases
                        xv_s = xin_pool.tile([128, 32, S], bf16, tag="x")
                        nc.sync.dma_start(out=xv_s, in_=xv_d[:, :, :])

                # ---- v projection (token-major) ----
                # Weight quarter-slabs are walked sequentially inside the
                # accumulation chain so each releases after its 32 matmuls.
                with (
                    tc.tile_pool(name="wv", bufs=3) as w_pool,
                    tc.tile_pool(name="psv", bufs=8, space="PSUM") as ps_pool,
                ):
                    for fc in range(8):
                        pss = [
                            ps_pool.tile([128, 512], f32, tag="ps", name="ps_v")
                            for _ in range(4)
                        ]
                        for qi in range(4):
                            wt = w_pool.tile([128, 8, 512], bf16, tag="w")
                            nc.sync.dma_start(
                                out=wt, in_=wv_d[fc, :, qi * 8 : (qi + 1) * 8, :]
                            )
                            for dl in range(8):
                                db = qi * 8 + dl
                                for tb in range(4):
                                    nc.tensor.matmul(
                                        pss[tb],
                                        xv_s[:, db, tb * 128 : (tb + 1) * 128],
                                        wt[:, dl, :],
                                        start=(db == 0),
                                        stop=(db == 31),
                                    )
                        for tb in range(4):
                            nc.vector.tensor_copy(
                                v_s[:, tb, fc * 4 : (fc + 1) * 4, :], pss[tb]
                            )

            # ---- attention ----
            # Softmax denominators accumulate per 16-head group so group
            # 0's reciprocal + normalize overlap group 1's matmuls.
            GH = 16
            with (
                tc.tile_pool(name="wexp", bufs=6) as wexp_pool,
                tc.tile_pool(name="rsm", bufs=1) as rpool,
                tc.tile_pool(name="ps_s", bufs=2, space="PSUM") as ps_s_pool,
                tc.tile_pool(name="ps_o", bufs=2, space="PSUM") as ps_o_pool,
                tc.tile_pool(name="ps_m", bufs=1, space="PSUM") as ps_m_pool,
                tc.tile_pool(name="ps_r", bufs=2, space="PSUM") as ps_r_pool,
            ):
                sums_t = [
                    ps_m_pool.tile([GH, S], f32, tag=f"sums{i}", name=f"sums{i}")
                    for i in range(2)
                ]

                def normalize_group(grp):
                    r_g = rpool.tile([GH, S], bf16, tag=f"r{grp}")
                    with nc.allow_low_precision(
                        reason="softmax reciprocal in bf16 is within tolerance"
                    ):
                        nc.vector.reciprocal(r_g, sums_t[grp])
                    for g in range(grp * GH, (grp + 1) * GH):
                        ps_rb = ps_r_pool.tile([128, S], f32, tag="rb")
                        nc.tensor.matmul(
                            ps_rb,
                            bsel_s[:, (g % GH) * 128 : (g % GH + 1) * 128],
                            r_g,
                            start=True,
                            stop=True,
                        )
                        nc.vector.tensor_mul(
                            v_s[:, :, g, :],
                            v_s[:, :, g, :],
                            ps_rb[:, :].rearrange("p (a b) -> p a b", a=4),
                        )

                for g in range(G):
                    grp, gl = divmod(g, GH)
                    ps_o = ps_o_pool.tile([128, S], f32, tag="o")
                    for kb in range(4):
                        ps_sc = ps_s_pool.tile([128, S], f32, tag="s")
                        nc.tensor.matmul(
                            ps_sc,
                            kT_s[:, g, kb * 128 : (kb + 1) * 128],
                            qT_s[:, g, :],
                            start=True,
                            stop=True,
                        )
                        wb = wexp_pool.tile([128, S], bf16, tag="w")
                        nc.scalar.activation(
                            wb, ps_sc, AF.Exp, bias=maskb[:, kb : kb + 1], scale=1.0
                        )
                        nc.tensor.matmul(
                            ps_o,
                            v_s[:, kb, g, :],
                            wb,
                            start=(kb == 0),
                            stop=(kb == 3),
                        )
                        nc.tensor.matmul(
                            sums_t[grp],
                            ssel_s[:, gl * GH : (gl + 1) * GH],
                            wb,
                            start=(gl == 0 and kb == 0),
                            stop=(gl == GH - 1 and kb == 3),
                        )
                    # unnormalized oT -> v_s space of head g (v now dead):
                    # v_s[:, tb, g, :] <- ps_o[:, tb*128:(tb+1)*128]
                    nc.vector.tensor_copy(
                        v_s[:, :, g, :],
                        ps_o[:, :].rearrange("p (a b) -> p a b", a=4),
                    )
                    if g == GH - 1:
                        normalize_group(0)
                normalize_group(1)

            # ---- y = attn @ Wo.T (token-major out) ----
            with (
                tc.tile_pool(name="wo", bufs=4) as w_pool,
                tc.tile_pool(name="psy", bufs=8, space="PSUM") as ps_pool,
                tc.tile_pool(name="yout", bufs=4) as y_pool,
            ):
                for fc in range(8):
                    pss = [
                        ps_pool.tile([128, 512], f32, tag="ps", name="ps_y")
                        for _ in range(4)
                    ]
                    for qi in range(4):
                        wt = w_pool.tile([128, 8, 512], bf16, tag="w")
                        nc.sync.dma_start(
                            out=wt, in_=wo_d[fc, :, qi * 8 : (qi + 1) * 8, :]
                        )
                        for dl in range(8):
                            db = qi * 8 + dl
                            for tb in range(4):
                                nc.tensor.matmul(
                                    pss[tb],
                                    v_s[:, tb, db, :],
                                    wt[:, dl, :],
                                    start=(db == 0),
                                    stop=(db == 31),
                                )
                    for tb in range(4):
                        yt = y_pool.tile([128, 512], f32, tag="y")
                        nc.vector.tensor_copy(yt, pss[tb])
                        nc.sync.dma_start(out=y_d[fc * 4 + tb, :, :], in_=yt)
    nc.compile()
    return nc


_NC_CACHE = None


def _get_program():
    global _NC_CACHE
    if _NC_CACHE is None:
        _NC_CACHE = build_program()
    return _NC_CACHE


def make_in_maps(query, key, value, mask, position_ids, Wq, Wk, Wv, Wo):
    bf16 = ml_dtypes.bfloat16

    def qk_tile(W):  # [4096,4096] -> [32 fb, 128 p, 32 db, 128 f]
        t = np.asarray(W, np.float32).astype(bf16)
        t = t.reshape(32, 128, 32, 128)  # [fb, f, db, p]
        return np.ascontiguousarray(t.transpose(0, 3, 2, 1))

    def vo_tile(W):  # [4096,4096] -> [8 fc, 128 p, 32 db, 512 f]
        t = np.asarray(W, np.float32).astype(bf16)
        t = t.reshape(8, 512, 32, 128)  # [fc, f, db, p]
        return np.ascontiguousarray(t.transpose(0, 3, 2, 1))

    def x_tile(x, scale=None):  # [512,4096] -> [128 p, 32 db, 512 t]
        x = np.asarray(x, np.float32)
        if scale is not None:
            x = x * scale
        t = x.astype(bf16).T.reshape(32, 128, S)  # [db, p, t]
        return np.ascontiguousarray(t.transpose(1, 0, 2))

    wq = qk_tile(np.asarray(Wq))
    wk = qk_tile(np.asarray(Wk))
    wv = vo_tile(np.asarray(Wv))
    wo = vo_tile(np.asarray(Wo))
    invf = (10000.0 ** (-np.arange(0, RD, 2, dtype=np.float32) / RD)).astype(
        np.float32
    )
    ssel = np.zeros((128, 16 * 16), bf16)
    for a in range(16):
        ssel[:, a * 16 + a] = 1
    bsel = np.zeros((16, 16 * 128), bf16)
    for a in range(16):
        bsel[a, a * 128 : (a + 1) * 128] = 1

    in_maps = []
    for b in range(NCORES):
        in_maps.append(
            {
                "xq": x_tile(query[b], ALPHA),
                "xk": x_tile(key[b]),
                "xv": x_tile(value[b]),
                "wq": wq,
                "wk": wk,
                "wv": wv,
                "wo": wo,
                "pos": np.ascontiguousarray(
                    np.asarray(position_ids[b], np.float32)
                ),
                "invf": invf,
                "maskin": np.ascontiguousarray(np.asarray(mask[b], np.int32)),
                "ssel": ssel,
                "bsel": bsel,
            }
        )
    return in_maps


def unshard_y(y_tiles):
    # [32, 128, 512] blocks (fc*4+tb) -> [512, 4096]
    return (
        y_tiles.reshape(8, 4, 128, 512)
        .transpose(1, 2, 0, 3)
        .reshape(S, D)
    )


def kernel(query, key, value, mask, position_ids, Wq, Wk, Wv, Wo):
    global LAST_RESULT
    nc = _get_program()
    in_maps = make_in_maps(
        query, key, value, mask, position_ids, Wq, Wk, Wv, Wo
    )
    res = run_bass_kernel_spmd(
        nc, in_maps, core_ids=list(range(NCORES)), trace=TRACE
    )
    LAST_RESULT = res
    out = np.stack(
        [unshard_y(np.asarray(res.results[b]["y"])) for b in range(NCORES)],
        axis=0,
    )
    return np.ascontiguousarray(out.astype(np.float32))
